# revision 10
# baseline (speedup 1.0000x reference)
# Block-local matmul kernel for Trainium2 (8 NeuronCores, SPMD) — v3.
#
# Problem: out[b, i*64+r, j*64+o] = sum_c x[b, i*64+r, j*64+c] * W[i*64+c, j*64+o]
# with B=4, M=K=N=4096, 64x64 blocks. Embarrassingly parallel over (i, j).
#
# Sharding: block-row axis i across the 8 cores; core p owns rows
# [512p, 512p+512) of x/out/W. No collectives.
#
# Key design points (evidence from cost-model timeline sims + HW probes):
#   - v1 was sequencer-bound (~3k PE instrs x ~71ns decode). v2 cut PE
#     instrs 3x via DMA-transpose loads + j-pair block-diag matmuls but
#     interleaved loads/stores on the same HWDGE ring — per-ring FIFO made
#     each transpose wait behind the previous strip's store (ping-pong
#     serialization, ~2.3us completion bubble per DMA).
#   - v3: DEDICATED queues (sync ring = transpose loads only, scalar ring
#     = stores only, SWDGE = W) and x2 batching (one 2MB transpose per
#     strip-PAIR, one 2MB store per pair, W in 2MB pieces) to halve the
#     per-DMA completion bubbles.
#   - x is host-cast to bf16 and host-prearranged to strip layout
#     [u, i, (t,r), K]: each transpose DMA src is contiguous [256, 4096].
#     The xbar writes wrong offsets into a strided mid-dim dst
#     (HW-verified) and matmul stationaries allow only ONE free dim, so
#     contiguity everywhere is load-bearing.
#   - W is host-prebuilt block-diag per j-pair: wd[c2, i, s, o2] with
#     W(i,2s) in quad (0,0) and W(i,2s+1) in quad (1,1) -> one matmul per
#     j-pair, 128-deep contraction, [128,128] contiguous PSUM writes.
#   - PSUM rule (HW): one matmul group per 2KB bank, readers may only
#     touch written bytes -> 4 banks per gather copy, [*, q, 0:128].
#   - Output stored as bf16 strip pairs, upcast + reassembled on host.
#
# Per-core HBM traffic: 16MB x + 8MB wd + 16MB out = 40MB -> ~112us at
# the ~358 GB/s per-core HBM limit.

import numpy as np

B = 4
M = K = N = 4096
NCORES = 8
RPC = M // NCORES  # 512 rows per core
NI = RPC // 64     # 8 i-blocks per core
NP = NI // 2       # 4 strip pairs per batch-pair
NJ = N // 64       # 64 j-blocks
NS = NJ // 2       # 32 j-pairs

_NC_CACHE = None


def _build_nc():
    import concourse.tile as tile
    from concourse import bacc, mybir

    f32 = mybir.dt.float32
    bf16 = mybir.dt.bfloat16

    nc = bacc.Bacc("TRN2", target_bir_lowering=False, debug=False,
                   num_devices=NCORES)
    # x in strip layout [u, i, (t, r), K]; pairs (i, i+1) are contiguous.
    x_d = nc.dram_tensor("x_shard", [2, NI, 128, K], bf16,
                         kind="ExternalInput")
    wd_d = nc.dram_tensor("wd_shard", [128, NI, NS, 128], bf16,
                          kind="ExternalInput")
    # out in pair layout [u, ip, (t, r), q, N].
    o_d = nc.dram_tensor("out_shard", [2, NP, 128, 2, N], bf16,
                         kind="ExternalOutput")

    with tile.TileContext(nc) as tc:
        with (
            tc.tile_pool(name="wd", bufs=1) as wdp,
            tc.tile_pool(name="at", bufs=3) as atp,
            tc.tile_pool(name="ob", bufs=2) as obp,
            tc.tile_pool(name="psO", bufs=2, space="PSUM") as psOp,
        ):
            # W in 4 pieces (2 i's each) so the first pair only waits for
            # piece 0. SWDGE only — keeps both HWDGE rings free.
            wd = wdp.tile([128, NI, NS, 128], bf16)
            for g in range(4):
                nc.gpsimd.dma_start(wd[:, 2 * g:2 * g + 2, :, :],
                                    wd_d.ap()[:, 2 * g:2 * g + 2, :, :])

            for u in range(2):         # batch pair (b in {2u, 2u+1})
                for ip in range(NP):   # strip pair (i = 2*ip + q)
                    # One 2MB transpose DMA per pair:
                    # atb[c2, s, 128q + tr] = x[u, 2ip+q, tr, 128s+c2].
                    atb = atp.tile([128, NS, 256], bf16, tag="at")
                    src = x_d.ap()[u, 2 * ip:2 * ip + 2]
                    nc.sync.dma_start_transpose(
                        atb[:], src.rearrange("a p k -> (a p) k"))

                    ob = obp.tile([128, 2, N], bf16, tag="ob")
                    for q in range(2):
                        i = 2 * ip + q
                        for g in range(8):       # groups of 4 j-pairs
                            psO = psOp.tile([128, 4, 512], f32, tag="psO")
                            for qq in range(4):
                                s = 4 * g + qq
                                nc.tensor.matmul(
                                    psO[:, qq, 0:128],
                                    atb[:, s, 128 * q:128 * q + 128],
                                    wd[:, i, s, :], start=True, stop=True)
                            dst = ob[:, q, 512 * g:512 * g + 512]
                            dst = dst.rearrange("p (c o) -> p c o", c=4)
                            if g % 2 == 0:
                                nc.vector.tensor_copy(dst, psO[:, :, 0:128])
                            else:
                                nc.scalar.copy(dst, psO[:, :, 0:128])

                    # One 2MB store per pair; dst outer dim = 128
                    # partitions so HWDGE sprays across SDMA engines.
                    nc.scalar.dma_start(o_d.ap()[u, ip], ob[:])

    nc.compile()
    return nc


def _get_nc():
    global _NC_CACHE
    if _NC_CACHE is None:
        _NC_CACHE = _build_nc()
    return _NC_CACHE


def prepare(x, weight):
    """Build (cached) nc and per-core input maps from full inputs."""
    import ml_dtypes

    bf16 = ml_dtypes.bfloat16
    x = np.asarray(x, dtype=np.float32)
    w = np.asarray(weight, dtype=np.float32)
    assert x.shape == (B, M, K) and w.shape == (K, N)
    x16 = x.astype(bf16)
    w16 = w.astype(bf16)

    nc = _get_nc()
    in_maps = []
    for c in range(NCORES):
        rows = slice(RPC * c, RPC * (c + 1))
        # Block-diag j-pair W: wd[c2, i, s, o2]; quad (0,0) = W(i, 2s),
        # quad (1,1) = W(i, 2s+1), off-diagonal quads zero.
        wc = w16[rows].reshape(NI, 64, NS, 2, 64)
        wd = np.zeros((128, NI, NS, 128), dtype=bf16)
        wd[0:64, :, :, 0:64] = wc[:, :, :, 0, :].transpose(1, 0, 2, 3)
        wd[64:128, :, :, 64:128] = wc[:, :, :, 1, :].transpose(1, 0, 2, 3)
        # Strip layout [u, i, (t, r), K]: b = 2u + t.
        xs = (x16[:, rows, :].reshape(2, 2, NI, 64, K)
              .transpose(0, 2, 1, 3, 4).reshape(2, NI, 128, K))
        in_maps.append({
            "x_shard": np.ascontiguousarray(xs),
            "wd_shard": wd,
        })
    return nc, in_maps


def kernel(x, weight):
    from concourse import bass_utils

    nc, in_maps = prepare(x, weight)
    res = bass_utils.run_bass_kernel_spmd(nc, in_maps,
                                          core_ids=list(range(NCORES)))
    out = np.empty((B, M, N), dtype=np.float32)
    for c in range(NCORES):
        # out_shard[u, ip, (t, r), q, n] -> out[2u+t, 512c+64(2ip+q)+r, n]
        arr = res.results[c]["out_shard"].reshape(2, NP, 2, 64, 2, N)
        out[:, RPC * c:RPC * (c + 1), :] = (
            arr.transpose(0, 2, 1, 4, 3, 5).reshape(B, RPC, N))
    return out


# revision 11
# speedup vs baseline: 1.6443x; 1.6443x over previous
# Block-local matmul kernel for Trainium2 (8 NeuronCores, SPMD) — v3.
#
# Problem: out[b, i*64+r, j*64+o] = sum_c x[b, i*64+r, j*64+c] * W[i*64+c, j*64+o]
# with B=4, M=K=N=4096, 64x64 blocks. Embarrassingly parallel over (i, j).
#
# Sharding: block-row axis i across the 8 cores; core p owns rows
# [512p, 512p+512) of x/out/W. No collectives.
#
# Key design points (evidence from cost-model timeline sims + HW probes):
#   - v1 was sequencer-bound (~3k PE instrs x ~71ns decode). v2 cut PE
#     instrs 3x via DMA-transpose loads + j-pair block-diag matmuls but
#     interleaved loads/stores on the same HWDGE ring — per-ring FIFO made
#     each transpose wait behind the previous strip's store (ping-pong
#     serialization, ~2.3us completion bubble per DMA).
#   - v3: DEDICATED queues (sync ring = transpose loads only, scalar ring
#     = stores only, SWDGE = W) and x2 batching (one 2MB transpose per
#     strip-PAIR, one 2MB store per pair, W in 2MB pieces) to halve the
#     per-DMA completion bubbles.
#   - x is host-cast to bf16 and host-prearranged to strip layout
#     [u, i, (t,r), K]: each transpose DMA src is contiguous [256, 4096].
#     The xbar writes wrong offsets into a strided mid-dim dst
#     (HW-verified) and matmul stationaries allow only ONE free dim, so
#     contiguity everywhere is load-bearing.
#   - W is host-prebuilt block-diag per j-pair: wd[c2, i, s, o2] with
#     W(i,2s) in quad (0,0) and W(i,2s+1) in quad (1,1) -> one matmul per
#     j-pair, 128-deep contraction, [128,128] contiguous PSUM writes.
#   - PSUM rule (HW): one matmul group per 2KB bank, readers may only
#     touch written bytes -> 4 banks per gather copy, [*, q, 0:128].
#   - Output stored as bf16 strip pairs, upcast + reassembled on host.
#
# Per-core HBM traffic: 16MB x + 8MB wd + 16MB out = 40MB -> ~112us at
# the ~358 GB/s per-core HBM limit.

import numpy as np

B = 4
M = K = N = 4096
NCORES = 8
RPC = M // NCORES  # 512 rows per core
NI = RPC // 64     # 8 i-blocks per core
NP = NI // 2       # 4 strip pairs per batch-pair
NJ = N // 64       # 64 j-blocks
NS = NJ // 2       # 32 j-pairs

_NC_CACHE = None


def _build_nc():
    import concourse.tile as tile
    from concourse import bacc, mybir

    f32 = mybir.dt.float32
    bf16 = mybir.dt.bfloat16

    nc = bacc.Bacc("TRN2", target_bir_lowering=False, debug=False,
                   num_devices=NCORES)
    # x in strip layout [u, i, (t, r), K]; pairs (i, i+1) are contiguous.
    x_d = nc.dram_tensor("x_shard", [2, NI, 128, K], bf16,
                         kind="ExternalInput")
    wd_d = nc.dram_tensor("wd_shard", [128, NI, NS, 128], bf16,
                          kind="ExternalInput")
    # out in pair layout [u, ip, (t, r), q, N].
    o_d = nc.dram_tensor("out_shard", [2, NP, 128, 2, N], bf16,
                         kind="ExternalOutput")

    with tile.TileContext(nc) as tc:
        with (
            tc.tile_pool(name="wd", bufs=1) as wdp,
            tc.tile_pool(name="at", bufs=3) as atp,
            tc.tile_pool(name="ob", bufs=2) as obp,
            tc.tile_pool(name="psO", bufs=2, space="PSUM") as psOp,
        ):
            # W in graduated pieces (1, 1, 2, 4 i's): the first matmuls
            # gate on a 1MB load instead of 4MB. SWDGE only — keeps both
            # HWDGE rings free.
            wd = wdp.tile([128, NI, NS, 128], bf16)
            for lo, hi in ((0, 1), (1, 2), (2, 4), (4, 8)):
                nc.gpsimd.dma_start(wd[:, lo:hi, :, :],
                                    wd_d.ap()[:, lo:hi, :, :])

            for u in range(2):         # batch pair (b in {2u, 2u+1})
                for h in range(2):     # strip quad (i = 4h + qs)
                    # One 4MB transpose DMA per quad:
                    # atb[c2, s, 128*qs + tr] = x[u, 4h+qs, tr, 128s+c2].
                    atb = atp.tile([128, NS, 512], bf16, tag="at")
                    src = x_d.ap()[u, 4 * h:4 * h + 4]
                    nc.sync.dma_start_transpose(
                        atb[:], src.rearrange("a p k -> (a p) k"))

                    for ipq in range(2):   # store pair within quad
                        ob = obp.tile([128, 2, N], bf16, tag="ob")
                        for q in range(2):
                            qs = 2 * ipq + q
                            i = 4 * h + qs
                            for g in range(8):   # groups of 4 j-pairs
                                psO = psOp.tile([128, 4, 512], f32,
                                                tag="psO")
                                for qq in range(4):
                                    s = 4 * g + qq
                                    nc.tensor.matmul(
                                        psO[:, qq, 0:128],
                                        atb[:, s, 128 * qs:128 * qs + 128],
                                        wd[:, i, s, :],
                                        start=True, stop=True)
                                dst = ob[:, q, 512 * g:512 * g + 512]
                                dst = dst.rearrange("p (c o) -> p c o", c=4)
                                if g % 2 == 0:
                                    nc.vector.tensor_copy(dst,
                                                          psO[:, :, 0:128])
                                else:
                                    nc.scalar.copy(dst, psO[:, :, 0:128])

                        # One 2MB store per pair; dst outer dim = 128
                        # partitions so HWDGE sprays across SDMA engines.
                        nc.scalar.dma_start(o_d.ap()[u, 2 * h + ipq], ob[:])

    nc.compile()
    return nc


def _get_nc():
    global _NC_CACHE
    if _NC_CACHE is None:
        _NC_CACHE = _build_nc()
    return _NC_CACHE


def prepare(x, weight):
    """Build (cached) nc and per-core input maps from full inputs."""
    import ml_dtypes

    bf16 = ml_dtypes.bfloat16
    x = np.asarray(x, dtype=np.float32)
    w = np.asarray(weight, dtype=np.float32)
    assert x.shape == (B, M, K) and w.shape == (K, N)
    x16 = x.astype(bf16)
    w16 = w.astype(bf16)

    nc = _get_nc()
    in_maps = []
    for c in range(NCORES):
        rows = slice(RPC * c, RPC * (c + 1))
        # Block-diag j-pair W: wd[c2, i, s, o2]; quad (0,0) = W(i, 2s),
        # quad (1,1) = W(i, 2s+1), off-diagonal quads zero.
        wc = w16[rows].reshape(NI, 64, NS, 2, 64)
        wd = np.zeros((128, NI, NS, 128), dtype=bf16)
        wd[0:64, :, :, 0:64] = wc[:, :, :, 0, :].transpose(1, 0, 2, 3)
        wd[64:128, :, :, 64:128] = wc[:, :, :, 1, :].transpose(1, 0, 2, 3)
        # Strip layout [u, i, (t, r), K]: b = 2u + t.
        xs = (x16[:, rows, :].reshape(2, 2, NI, 64, K)
              .transpose(0, 2, 1, 3, 4).reshape(2, NI, 128, K))
        in_maps.append({
            "x_shard": np.ascontiguousarray(xs),
            "wd_shard": wd,
        })
    return nc, in_maps


def kernel(x, weight):
    from concourse import bass_utils

    nc, in_maps = prepare(x, weight)
    res = bass_utils.run_bass_kernel_spmd(nc, in_maps,
                                          core_ids=list(range(NCORES)))
    out = np.empty((B, M, N), dtype=np.float32)
    for c in range(NCORES):
        # out_shard[u, ip, (t, r), q, n] -> out[2u+t, 512c+64(2ip+q)+r, n]
        arr = res.results[c]["out_shard"].reshape(2, NP, 2, 64, 2, N)
        out[:, RPC * c:RPC * (c + 1), :] = (
            arr.transpose(0, 2, 1, 4, 3, 5).reshape(B, RPC, N))
    return out
